# revision 1
# baseline (speedup 1.0000x reference)
"""BatchHardTripletLoss kernel for 8 Trainium2 NeuronCores.

Math (matches the jax reference):
  dist2[i,j] = |e1_i|^2 + |e2_j|^2 - 2 e1.e2 + 2*eps*(s1_i - s2_j) + D*eps^2
             = a[i] + (b[j] - 2*G[i,j])
  pos_max[i] = sqrt(clip(a[i] + max_{j in pos}(b[j] - 2 G[i,j]), 0))
  neg_min[i] = sqrt(clip(a[i] + min_{j in neg}(b[j] - 2 G[i,j]), 0))
  loss = mean over pos anchors of relu(pos_max - neg_min + margin)

Device strategy (data parallel over emb1 rows, hint-compliant):
  - Host: sort emb2 rows so target==1 rows come first (masks become
    contiguous column ranges), precompute a[i]/b[j] row stats, transpose
    both embeddings to [D=128, rows] layout, cast to bf16 (rel err of the
    final loss ~1e-5, verified), scale emb1 by -2.
  - Each core gets 1024 anchor rows: computes G-blocks on TensorE
    (bf16, K=128, N=512 per matmul into fp32 PSUM) and reduces with the
    fused DVE op tensor_tensor_reduce: accum = reduce_minmax(psum + bias)
    chained across column groups via the scalar initial value.
  - Device output per core: [128, 16] fp32 = per-i-tile max/min partials.
  - Host: adds a[i], sqrt, margin/relu, weighted mean (O(B) work).
"""

import os
import sys

for _p in ("/opt/trn_rl_repo",):
    if _p not in sys.path:
        sys.path.insert(0, _p)

import numpy as np
import ml_dtypes

EPS = 1e-6
MARGIN = 0.2
B = 8192
D = 128
NCORES = 8
SH = B // NCORES      # anchors per core
NIT = SH // 128       # i-tiles of 128 anchors per core
GW = 2048             # candidate-group width = 4 PSUM banks
NG = B // GW
PSUM_BUFS = 2
BIG = 1.0e30

_programs = {}
LAST_RESULTS = None   # BassKernelResults of the most recent run (for profiling)


def _build_program(k: int):
    """Bass program for one core; k = number of positive candidates
    (boundary between the max- and min-reduced column ranges)."""
    import concourse.bacc as bacc
    import concourse.tile as tile
    from concourse import mybir

    f32 = mybir.dt.float32
    bf16 = mybir.dt.bfloat16
    AOT = mybir.AluOpType

    nc = bacc.Bacc(None)
    e1t = nc.declare_dram_parameter("e1t", [D, SH], bf16, isOutput=False)
    e2t = nc.declare_dram_parameter("e2t", [D, B], bf16, isOutput=False)
    tailw = nc.declare_dram_parameter("tailw", [16, SH], bf16, isOutput=False)
    trhs = nc.declare_dram_parameter("trhs", [16, B], bf16, isOutput=False)
    outp = nc.declare_dram_parameter("out", [128, 2 * NIT], f32, isOutput=True)

    # per-group reduction segments: (lo, hi, is_pos) in global column coords
    def group_segs(g):
        glo, ghi = g * GW, (g + 1) * GW
        segs = []
        if glo < k:
            segs.append((glo, min(ghi, k), True))
        if ghi > k:
            segs.append((max(glo, k), ghi, False))
        return segs

    n_pos_segs = sum(1 for g in range(NG) for s in group_segs(g) if s[2])
    n_neg_segs = sum(1 for g in range(NG) for s in group_segs(g) if not s[2])

    with tile.TileContext(nc) as tc:
        with (
            tc.tile_pool(name="const", bufs=1) as cpool,
            tc.tile_pool(name="e2p", bufs=NG) as e2pool,
            tc.tile_pool(name="ps", bufs=PSUM_BUFS, space="PSUM") as pspool,
            tc.tile_pool(name="red", bufs=4) as redpool,
        ):
            e1sb = cpool.tile([D, SH], bf16, tag="e1sb")
            nc.sync.dma_start(e1sb[:], e1t[:])
            twsb = cpool.tile([128, SH], bf16, tag="twsb")
            trsb = cpool.tile([128, B], bf16, tag="trsb")
            for s in range(4):
                nc.sync.dma_start(twsb[32 * s:32 * s + 4, :], tailw[4 * s:4 * s + 4, :])
            outsb = cpool.tile([128, 2 * NIT], f32, tag="outsb")

            e2sb = []
            for g in range(NG):
                e2c = e2pool.tile([D, GW], bf16, tag="e2c")
                nc.sync.dma_start(e2c[:], e2t[:, g * GW:(g + 1) * GW])
                e2sb.append(e2c)
                if g == 0:
                    for s in range(4):
                        nc.sync.dma_start(
                            trsb[32 * s:32 * s + 4, :], trhs[4 * s:4 * s + 4, :]
                        )

            for it in range(NIT):
                icols = slice(it * 128, (it + 1) * 128)
                w = e1sb[0:126, icols]
                posb = redpool.tile([128, n_pos_segs], f32, tag="posb")
                negb = redpool.tile([128, n_neg_segs], f32, tag="negb")
                ip = 0
                ineg = 0
                for g in range(NG):
                    ps = pspool.tile([128, GW], f32, tag="ps")
                    # K=126 mains (embedding dims 0..125)
                    for s in range(GW // 512):
                        nc.tensor.matmul(
                            ps[:, s * 512:(s + 1) * 512],
                            w,
                            e2sb[g][0:126, s * 512:(s + 1) * 512],
                            start=True,
                            stop=False,
                        )
                    # K=4 tails (dims 126,127 + bias hi/lo), 4-way
                    # row-packed so the four sub-tiles run concurrently
                    for s in range(GW // 512):
                        j0 = g * GW + s * 512
                        nc.tensor.matmul(
                            ps[:, s * 512:(s + 1) * 512],
                            twsb[32 * s:32 * s + 4, icols],
                            trsb[32 * s:32 * s + 4, j0:j0 + 512],
                            start=False,
                            stop=True,
                            tile_position=(32 * s, 0),
                        )
                    for lo, hi, is_pos in group_segs(g):
                        if is_pos:
                            dst = posb[:, ip:ip + 1]
                            ip += 1
                        else:
                            dst = negb[:, ineg:ineg + 1]
                            ineg += 1
                        nc.vector.tensor_reduce(
                            dst,
                            ps[:, lo - g * GW:hi - g * GW],
                            axis=mybir.AxisListType.X,
                            op=AOT.max if is_pos else AOT.min,
                        )
                nc.vector.tensor_reduce(
                    outsb[:, it:it + 1], posb[:],
                    axis=mybir.AxisListType.X, op=AOT.max,
                )
                nc.vector.tensor_reduce(
                    outsb[:, NIT + it:NIT + it + 1], negb[:],
                    axis=mybir.AxisListType.X, op=AOT.min,
                )
            nc.sync.dma_start(outp[:], outsb[:])
    nc.compile()
    return nc


def _host_prep(emb1, emb2, target):
    tpos = target == 1
    k = int(tpos.sum())
    perm = np.concatenate([np.nonzero(tpos)[0], np.nonzero(~tpos)[0]])
    e2s = emb2[perm]
    e2d = e2s.astype(np.float64)
    e1d = emb1.astype(np.float64)
    b = (e2d * e2d).sum(1) - (2.0 * EPS) * e2d.sum(1)
    a = (e1d * e1d).sum(1) + (2.0 * EPS) * e1d.sum(1) + D * EPS * EPS
    e1tb = np.ascontiguousarray((-2.0 * emb1).T.astype(ml_dtypes.bfloat16))
    e2tb = np.ascontiguousarray(e2s.T.astype(ml_dtypes.bfloat16))
    bhi = b.astype(np.float32).astype(ml_dtypes.bfloat16)
    blo = (b.astype(np.float32) - bhi.astype(np.float32)).astype(ml_dtypes.bfloat16)
    # K=4 tail operands; on device row 4s+r lands at partition 32s+r so the
    # four 512-wide sub-tiles of a group can row-pack on the PE array.
    tailw = np.zeros((16, B), dtype=ml_dtypes.bfloat16)
    trhs = np.zeros((16, B), dtype=ml_dtypes.bfloat16)
    one = np.ones(B, dtype=ml_dtypes.bfloat16)
    for s in range(4):
        tailw[4 * s + 0] = e1tb[126]
        tailw[4 * s + 1] = e1tb[127]
        tailw[4 * s + 2] = one
        tailw[4 * s + 3] = one
        trhs[4 * s + 0] = e2tb[126]
        trhs[4 * s + 1] = e2tb[127]
        trhs[4 * s + 2] = bhi
        trhs[4 * s + 3] = blo
    return k, a, e1tb, e2tb, tailw, trhs, tpos


def _host_finish(a, Mp, mn, tpos, k):
    pos2 = np.clip(a + Mp.astype(np.float64), 0.0, None)
    neg2 = np.clip(a + mn.astype(np.float64), 0.0, None)
    per = np.clip(np.sqrt(pos2) - np.sqrt(neg2) + MARGIN, 0.0, None)
    return np.float32((per * tpos).sum() / k)


def _numpy_fallback(emb1, emb2, target):
    # exact reference recomputation in numpy (degenerate target mixes)
    e1 = emb1.astype(np.float64)
    e2 = emb2.astype(np.float64)
    sq = (
        (e1 * e1).sum(1)[:, None]
        + (e2 * e2).sum(1)[None, :]
        - 2.0 * (e1 @ e2.T)
        + 2.0 * EPS * (e1.sum(1)[:, None] - e2.sum(1)[None, :])
        + D * EPS * EPS
    )
    dist = np.sqrt(np.clip(sq, 0.0, None))
    pos = target == 1
    neg = target == 0
    pos_max = np.where(pos[None, :], dist, -np.inf).max(1)
    neg_min = np.where(neg[None, :], dist, np.inf).min(1)
    per = np.maximum(pos_max - neg_min + MARGIN, 0.0)
    w = pos.astype(np.float64)
    return np.float32((per * w).sum() / w.sum())


def kernel(emb1, emb2, target):
    global LAST_RESULTS
    emb1 = np.asarray(emb1, dtype=np.float32)
    emb2 = np.asarray(emb2, dtype=np.float32)
    target = np.asarray(target)
    assert emb1.shape == (B, D) and emb2.shape == (B, D)

    k = int((target == 1).sum())
    if k == 0 or k == B:
        return _numpy_fallback(emb1, emb2, target)

    k, a, e1tb, e2tb, tailw, trhs, tpos = _host_prep(emb1, emb2, target)

    nc = _programs.get(k)
    if nc is None:
        nc = _build_program(k)
        _programs[k] = nc

    from concourse.bass_utils import run_bass_kernel_spmd

    in_maps = [
        {
            "e1t": np.ascontiguousarray(e1tb[:, c * SH:(c + 1) * SH]),
            "e2t": e2tb,
            "tailw": np.ascontiguousarray(tailw[:, c * SH:(c + 1) * SH]),
            "trhs": trhs,
        }
        for c in range(NCORES)
    ]
    res = run_bass_kernel_spmd(nc, in_maps, core_ids=list(range(NCORES)))
    LAST_RESULTS = res

    Mp = np.concatenate(
        [np.asarray(res.results[c]["out"])[:, :NIT].T.reshape(-1) for c in range(NCORES)]
    )
    mn = np.concatenate(
        [np.asarray(res.results[c]["out"])[:, NIT:].T.reshape(-1) for c in range(NCORES)]
    )
    return _host_finish(a, Mp, mn, tpos, k)



# revision 3
# speedup vs baseline: 1.8279x; 1.8279x over previous
"""BatchHardTripletLoss kernel for 8 Trainium2 NeuronCores.

Math (matches the jax reference):
  dist2[i,j] = |e1_i|^2 + |e2_j|^2 - 2 e1.e2 + 2*eps*(s1_i - s2_j) + D*eps^2
             = a[i] + v[i,j],   v[i,j] = b[j] - 2*G[i,j]
  pos_max[i] = sqrt(max_{j in pos} dist2), neg_min[i] = sqrt(min_{j in neg})
  loss = mean over pos anchors of relu(pos_max - neg_min + margin)

Only rows with target[i]==1 contribute, so the device computes pos anchors
only (~k/8 rows per core); the k % 1024 leftover anchors are done exactly on
the host (tiny numpy job).

Device strategy (data parallel over pos-anchor rows, 8 cores):
  - Host sorts emb2 columns pos-first, transposes to [D, cols] bf16, scales
    emb1 by -2; bias b[j] enters PSUM via K=4 row-packed tail matmuls
    (dims 126/127 + bf16 hi/lo split of b) exactly like the mains.
  - Per (i-tile of 128 anchors, column group of 1024): TensorE computes
    v into PSUM (4 bufs x 2 banks); groups are issued pos/neg interleaved
    so both consumer engines run concurrently and the PE never idles long
    (keeps the HAM activity clock-gate at 2.4 GHz).
  - Consumers: neg columns -> VectorE exact tensor_reduce(min).
    pos columns -> ScalarE activation(Exp, scale=T, bias=per-row AP,
    accum_out): sum of exp(T*(dist2 - K_i)) is a log-sum-exp max that the
    host finishes with one ln(); the per-row offset K_i = a_i + c1 + c2*
    sqrt(a_i) + U keeps every row inside the f32 exp window.  A slice of
    the neg side right after the pos/neg boundary is likewise LSE-min'ed
    on ScalarE to balance engine load.  Rows outside the window (none on
    real data) are detected on the host and recomputed exactly.
  - Host: ln/sqrt/relu/mean in f64 (O(k) work).
"""

import sys

for _p in ("/opt/trn_rl_repo",):
    if _p not in sys.path:
        sys.path.insert(0, _p)

import numpy as np
import ml_dtypes

EPS = 1e-6
MARGIN = 0.2
B = 8192
D = 128
NCORES = 8
GW = 1024             # candidate-group width = 2 PSUM banks
NG = B // GW
PSUM_BUFS = 4
ISSUE = [4, 0, 5, 1, 6, 2, 7, 3]   # pos/neg interleaved group issue order
WARMUP_MM = 6         # dummy matmuls to warm the HAM clock gate

# log-sum-exp calibration (fit to randn(B,128) stats; host detector +
# exact-row fallback covers anything outside the window)
T = 1.0
BIASP = lambda sa: -(131.26 + 22.0 + 10.20 * sa) * T   # pos/max side
BIASN = lambda sa: (126.17 - 17.0 - 8.64 * sa) * T     # neg/min side
XNEG = 344            # neg cols right after k handled by LSE on ScalarE

_programs = {}
LAST_RESULTS = None   # BassKernelResults of the most recent run (for profiling)


def _plan_segments(k):
    """Per group: list of (kind, lo, hi) in global sorted-column coords.
    kind: 'ap' = Act pos (LSE max), 'an' = Act neg (LSE min), 'dv' = DVE min."""
    plans = []
    for g in range(NG):
        glo, ghi = g * GW, (g + 1) * GW
        ops = []
        if min(ghi, k) > glo:
            ops.append(("ap", glo, min(ghi, k)))
        nl = max(glo, k)
        if ghi > nl:
            xh = max(nl, min(ghi, min(k + XNEG, B)))
            if xh > nl:
                ops.append(("an", nl, xh))
            if ghi > xh:
                ops.append(("dv", xh, ghi))
        plans.append(ops)
    return plans


def _build_program(k, kp):
    import concourse.bacc as bacc
    import concourse.tile as tile
    from concourse import mybir

    f32 = mybir.dt.float32
    bf16 = mybir.dt.bfloat16
    AOT = mybir.AluOpType
    AFT = mybir.ActivationFunctionType

    SH = kp // NCORES
    NIT = SH // 128
    plans = _plan_segments(k)
    nslot = sum(len(ops) for ops in plans)

    nc = bacc.Bacc(None)
    e1t = nc.declare_dram_parameter("e1t", [D, SH], bf16, isOutput=False)
    e2t = nc.declare_dram_parameter("e2t", [D, B], bf16, isOutput=False)
    tailw = nc.declare_dram_parameter("tailw", [16, SH], bf16, isOutput=False)
    trhs = nc.declare_dram_parameter("trhs", [16, B], bf16, isOutput=False)
    biasp = nc.declare_dram_parameter("biasp", [128, NIT], f32, isOutput=False)
    biasn = nc.declare_dram_parameter("biasn", [128, NIT], f32, isOutput=False)
    outp = nc.declare_dram_parameter("out", [128, NIT * nslot], f32, isOutput=True)

    with tile.TileContext(nc) as tc:
        with (
            tc.tile_pool(name="const", bufs=1) as cpool,
            tc.tile_pool(name="e2p", bufs=NG) as e2pool,
            tc.tile_pool(name="ps", bufs=PSUM_BUFS, space="PSUM") as pspool,
        ):
            e1sb = cpool.tile([D, SH], bf16, tag="e1sb")
            nc.sync.dma_start(e1sb[:], e1t[:])
            twsb = cpool.tile([128, SH], bf16, tag="twsb")
            for s in range(4):
                nc.sync.dma_start(twsb[32 * s:32 * s + 4, :], tailw[4 * s:4 * s + 4, :])
            bpsb = cpool.tile([128, NIT], f32, tag="bpsb")
            nc.sync.dma_start(bpsb[:], biasp[:])
            bnsb = cpool.tile([128, NIT], f32, tag="bnsb")
            nc.sync.dma_start(bnsb[:], biasn[:])
            trsb = cpool.tile([128, B], bf16, tag="trsb")
            outsb = cpool.tile([128, NIT * nslot], f32, tag="outsb")
            scr = cpool.tile([128, GW], f32, tag="scr")

            # kick the Exp table load early so it overlaps input DMA
            nc.scalar.activation(scr[:, 0:1], bpsb[:, 0:1], AFT.Exp)

            # warm the PE activity window with throwaway matmuls
            if WARMUP_MM:
                psw = pspool.tile([128, GW], f32, tag="ps")
                for s in range(WARMUP_MM):
                    nc.tensor.matmul(
                        psw[:, 0:512], e1sb[0:126, 0:128], e1sb[0:126, 0:512],
                        start=True, stop=True,
                    )

            # e2/trhs chunks, first two in issue order then the rest
            e2sb = [None] * NG
            def load_chunk(g):
                e2c = e2pool.tile([D, GW], bf16, tag=f"e2c{g}")
                nc.sync.dma_start(e2c[:], e2t[:, g * GW:(g + 1) * GW])
                e2sb[g] = e2c
            load_chunk(ISSUE[0])
            load_chunk(ISSUE[1])
            for s in range(4):
                nc.sync.dma_start(trsb[32 * s:32 * s + 4, :], trhs[4 * s:4 * s + 4, :])
            for g in ISSUE[2:]:
                load_chunk(g)

            slot = 0
            for it in range(NIT):
                icols = slice(it * 128, (it + 1) * 128)
                w = e1sb[0:126, icols]
                # issue groups pairwise: mains(gA), mains(gB), 4 packed tails
                for pi in range(0, NG, 2):
                    pair = (ISSUE[pi], ISSUE[pi + 1])
                    pstiles = []
                    for g in pair:
                        ps = pspool.tile([128, GW], f32, tag="ps")
                        for s in range(GW // 512):
                            nc.tensor.matmul(
                                ps[:, s * 512:(s + 1) * 512],
                                w,
                                e2sb[g][0:126, s * 512:(s + 1) * 512],
                                start=True,
                                stop=False,
                            )
                        pstiles.append(ps)
                    # K=4 tails (dims 126,127 + bias hi/lo), row-packed so the
                    # four sub-tiles run concurrently across the pair
                    for si in range(4):
                        g = pair[si // 2]
                        ps = pstiles[si // 2]
                        s = si % 2
                        j0 = g * GW + s * 512
                        nc.tensor.matmul(
                            ps[:, s * 512:(s + 1) * 512],
                            twsb[32 * si:32 * si + 4, icols],
                            trsb[32 * si:32 * si + 4, j0:j0 + 512],
                            start=False,
                            stop=True,
                            tile_position=(32 * si, 0),
                        )
                    # consumers
                    for g, ps in zip(pair, pstiles):
                        for kind, lo, hi in plans[g]:
                            dst = outsb[:, slot:slot + 1]
                            slot += 1
                            l, h = lo - g * GW, hi - g * GW
                            if kind == "dv":
                                nc.vector.tensor_reduce(
                                    dst, ps[:, l:h],
                                    axis=mybir.AxisListType.X, op=AOT.min,
                                )
                            else:
                                nc.scalar.activation(
                                    scr[:, 0:h - l], ps[:, l:h], AFT.Exp,
                                    bias=(bpsb if kind == "ap" else bnsb)[:, it:it + 1],
                                    scale=(T if kind == "ap" else -T),
                                    accum_out=dst,
                                )
            nc.sync.dma_start(outp[:], outsb[:])
    nc.compile()
    return nc


def _host_prep(emb1, emb2, target, kp):
    tpos = target == 1
    k = int(tpos.sum())
    pos_idx = np.nonzero(tpos)[0]
    perm = np.concatenate([pos_idx, np.nonzero(~tpos)[0]])
    e2s = emb2[perm]
    e2d = e2s.astype(np.float64)
    b = (e2d * e2d).sum(1) - (2.0 * EPS) * e2d.sum(1)
    e1dev = emb1[pos_idx[:kp]]
    e1d = e1dev.astype(np.float64)
    a = (e1d * e1d).sum(1) + (2.0 * EPS) * e1d.sum(1) + D * EPS * EPS
    e1tb = np.ascontiguousarray((-2.0 * e1dev).T.astype(ml_dtypes.bfloat16))
    e2tb = np.ascontiguousarray(e2s.T.astype(ml_dtypes.bfloat16))
    bhi = b.astype(np.float32).astype(ml_dtypes.bfloat16)
    blo = (b.astype(np.float32) - bhi.astype(np.float32)).astype(ml_dtypes.bfloat16)
    # K=4 tail operands; on device row 4s+r lands at partition 32s+r so the
    # four 512-wide sub-tiles of a pair can row-pack on the PE array.
    tailw = np.zeros((16, kp), dtype=ml_dtypes.bfloat16)
    trhs = np.zeros((16, B), dtype=ml_dtypes.bfloat16)
    one = np.ones(B, dtype=ml_dtypes.bfloat16)
    for s in range(4):
        tailw[4 * s + 0] = e1tb[126]
        tailw[4 * s + 1] = e1tb[127]
        tailw[4 * s + 2] = one[:kp]
        tailw[4 * s + 3] = one[:kp]
        trhs[4 * s + 0] = e2tb[126]
        trhs[4 * s + 1] = e2tb[127]
        trhs[4 * s + 2] = bhi
        trhs[4 * s + 3] = blo
    sa = np.sqrt(a)
    bp = BIASP(sa).astype(np.float32)   # exp arg = T*v + bp  (max side)
    bn = BIASN(sa).astype(np.float32)   # exp arg = -T*v + bn (min side)
    return k, a, b, e2d, pos_idx, e1tb, e2tb, tailw, trhs, bp, bn


def _exact_rows(e1rows, e2d, b, k):
    """Exact f64 pos_max2/neg_min2 for a handful of anchor rows."""
    e1d = e1rows.astype(np.float64)
    av = (e1d * e1d).sum(1) + (2.0 * EPS) * e1d.sum(1) + D * EPS * EPS
    d2 = av[:, None] + b[None, :] - 2.0 * (e1d @ e2d.T)
    return d2[:, :k].max(1), d2[:, k:].min(1)


def _numpy_fallback(emb1, emb2, target):
    e1 = emb1.astype(np.float64)
    e2 = emb2.astype(np.float64)
    sq = (
        (e1 * e1).sum(1)[:, None]
        + (e2 * e2).sum(1)[None, :]
        - 2.0 * (e1 @ e2.T)
        + 2.0 * EPS * (e1.sum(1)[:, None] - e2.sum(1)[None, :])
        + D * EPS * EPS
    )
    dist = np.sqrt(np.clip(sq, 0.0, None))
    pos = target == 1
    neg = target == 0
    pos_max = np.where(pos[None, :], dist, -np.inf).max(1)
    neg_min = np.where(neg[None, :], dist, np.inf).min(1)
    per = np.maximum(pos_max - neg_min + MARGIN, 0.0)
    w = pos.astype(np.float64)
    return np.float32((per * w).sum() / w.sum())


def kernel(emb1, emb2, target):
    global LAST_RESULTS
    emb1 = np.asarray(emb1, dtype=np.float32)
    emb2 = np.asarray(emb2, dtype=np.float32)
    target = np.asarray(target)
    assert emb1.shape == (B, D) and emb2.shape == (B, D)

    k = int((target == 1).sum())
    kp = (k // (NCORES * 128)) * (NCORES * 128)
    if kp == 0 or k == B:
        return _numpy_fallback(emb1, emb2, target)

    k, a, b, e2d, pos_idx, e1tb, e2tb, tailw, trhs, bp, bn = _host_prep(
        emb1, emb2, target, kp
    )

    nc = _programs.get((k, kp))
    if nc is None:
        nc = _build_program(k, kp)
        _programs[(k, kp)] = nc

    from concourse.bass_utils import run_bass_kernel_spmd

    SH = kp // NCORES
    NIT = SH // 128
    in_maps = [
        {
            "e1t": np.ascontiguousarray(e1tb[:, c * SH:(c + 1) * SH]),
            "e2t": e2tb,
            "tailw": np.ascontiguousarray(tailw[:, c * SH:(c + 1) * SH]),
            "trhs": trhs,
            "biasp": np.ascontiguousarray(
                bp[c * SH:(c + 1) * SH].reshape(NIT, 128).T
            ),
            "biasn": np.ascontiguousarray(
                bn[c * SH:(c + 1) * SH].reshape(NIT, 128).T
            ),
        }
        for c in range(NCORES)
    ]
    res = run_bass_kernel_spmd(nc, in_maps, core_ids=list(range(NCORES)))
    LAST_RESULTS = res

    # ---- host reconstruction (all f64) ----
    plans = _plan_segments(k)
    nslot = sum(len(ops) for ops in plans)
    # per-i-tile slot ids by kind, in device issue order
    slot_ap, slot_an, slot_dv = [], [], []
    si = 0
    for pi in range(0, NG, 2):
        for g in (ISSUE[pi], ISSUE[pi + 1]):
            for kind, lo, hi in plans[g]:
                (slot_ap if kind == "ap" else slot_an if kind == "an" else slot_dv
                 ).append(si)
                si += 1
    assert si == nslot

    Sp = np.zeros(kp)
    Sn = np.zeros(kp)
    vmin = np.full(kp, np.inf)
    for c in range(NCORES):
        out = np.asarray(res.results[c]["out"]).astype(np.float64)  # [128, NIT*nslot]
        for it in range(NIT):
            rows = slice(c * SH + it * 128, c * SH + (it + 1) * 128)
            blk = out[:, it * nslot:(it + 1) * nslot]
            for s in slot_ap:
                Sp[rows] += blk[:, s]
            for s in slot_an:
                Sn[rows] += blk[:, s]
            for s in slot_dv:
                vmin[rows] = np.minimum(vmin[rows], blk[:, s])

    bp64 = bp.astype(np.float64)
    bn64 = bn.astype(np.float64)
    with np.errstate(divide="ignore", invalid="ignore"):
        pm2 = a + (np.log(Sp) - bp64) / T
        nm2_lse = np.where(Sn > 0, a + (bn64 - np.log(Sn)) / T, np.inf)
    nm2 = np.minimum(a + vmin, nm2_lse)

    # detector: rows where the LSE left the reliable window -> exact redo
    bad = (~np.isfinite(Sp)) | (Sp <= 0) | (np.log(np.maximum(Sp, 1e-300)) < -60.0)
    bad |= ~np.isfinite(Sn)
    bad |= ~np.isfinite(nm2)
    if bad.any():
        idx = np.nonzero(bad)[0]
        pmx, nmx = _exact_rows(emb1[pos_idx[idx]], e2d, b, k)
        pm2[idx] = pmx
        nm2[idx] = nmx

    per = np.maximum(
        np.sqrt(np.clip(pm2, 0.0, None)) - np.sqrt(np.clip(nm2, 0.0, None)) + MARGIN,
        0.0,
    )
    total = per.sum()

    if k > kp:  # leftover pos anchors, exact on host
        pmx, nmx = _exact_rows(emb1[pos_idx[kp:k]], e2d, b, k)
        total += np.maximum(
            np.sqrt(np.clip(pmx, 0.0, None)) - np.sqrt(np.clip(nmx, 0.0, None)) + MARGIN,
            0.0,
        ).sum()

    return np.float32(total / k)
